# revision 26
# baseline (speedup 1.0000x reference)
"""MoE FFN (8 experts, top-2) Trainium2 kernel.

Strategy (expert-parallel with quarter-strip load balancing):
  - Host computes the gate (softmax + top-2 + renormalize) in float64.
  - Experts are sorted by routed-token count (padded to 8) and paired
    (heaviest with lightest-of-pair): pair g defines slot width
    W_g = padded count of the heavier expert.  Every core runs 4 slots;
    slot g holds ONE quarter (11 of 44 h-tiles) of one expert of pair g
    (cores 0-3: the heavy expert's quarters 0-3, cores 4-7: the light
    expert's).  Each (expert, h-tile) lives on exactly one core, so
    weight DMA traffic is unchanged while per-core matmul work drops
    from 44*max_e N_e to 11*(W_0+W_1+W_2+W_3).
  - Per slot the core runs the quarter FFN on the expert's tokens:
        H^T = silu(W1q^T x^T) * (W3q^T x^T)      (1408 hidden rows)
        y^T_partial = W2q^T H^T                  (partial over hidden)
    H^T staged through DRAM in bf16.  Host sums the 4 partial y per
    expert and applies the gate weights.

All matmul operands bf16 (fp32 PSUM accumulation).  silu via the ACT
Silu table.  DMA queues: w1+w2+y on sync, w3+strips on scalar (HWDGE),
x and h-stores on gpsimd (SWDGE), so no queue's issue rate saturates.
"""

import math
from contextlib import ExitStack

import ml_dtypes
import numpy as np

P = 128
D_MODEL = 2048
HIDDEN = 5632
N_EXPERTS = 8
TOP_K = 2
N_CORES = 8

HC = HIDDEN // P          # 44 h-tiles
QT = HC // 4              # 11 h-tiles per slot (quarter)
QH = QT * P               # 1408 hidden per quarter
DC = D_MODEL // P         # 16 contraction tiles
D_SPAN = 1024             # phase-2 resident W2 span along d_model
DT_SWEEP = 4              # d-tiles accumulated concurrently in phase 2
# phase-1 weight groups within a quarter: (tile_start, n_tiles)
HGROUPS = [(0, 2), (2, 2), (4, 2), (6, 2), (8, 2), (10, 1)]

_prog_cache: dict[tuple, object] = {}


def _chunk_list(n_pad: int, max_chunk: int = 512) -> list[tuple[int, int]]:
    """Split [0, n_pad) into near-equal chunks <= max_chunk, multiples of 8."""
    assert n_pad % 8 == 0
    k = math.ceil(n_pad / max_chunk)
    base = (n_pad // k) // 8 * 8
    sizes = [base] * k
    extra = n_pad - base * k
    i = 0
    while extra > 0:
        sizes[i] += 8
        extra -= 8
        i = (i + 1) % k
    out = []
    n0 = 0
    for s in sizes:
        out.append((n0, s))
        n0 += s
    return out


def _build_program(widths: tuple):
    import concourse.bacc as bacc
    import concourse.mybir as mybir
    import concourse.tile as tile

    f32 = mybir.dt.float32
    bf16 = mybir.dt.bfloat16
    Silu = mybir.ActivationFunctionType.Silu
    mult = mybir.AluOpType.mult

    W0 = widths[0]  # widths sorted desc; W0 is the max
    chunk_lists = [_chunk_list(w) for w in widths]

    nc = bacc.Bacc(
        "TRN2",
        target_bir_lowering=False,
        debug=False,
        enable_asserts=False,
        num_devices=N_CORES,
    )
    xT, w1, w3, w2, hbuf, yT = [], [], [], [], [], []
    for g in range(4):
        W = widths[g]
        xT.append(nc.dram_tensor(f"xT{g}", [D_MODEL, W], bf16, kind="ExternalInput").ap())
        w1.append(nc.dram_tensor(f"w1{g}", [D_MODEL, QH], bf16, kind="ExternalInput").ap())
        w3.append(nc.dram_tensor(f"w3{g}", [D_MODEL, QH], bf16, kind="ExternalInput").ap())
        w2.append(nc.dram_tensor(f"w2{g}", [QH, D_MODEL], bf16, kind="ExternalInput").ap())
        hbuf.append(nc.dram_tensor(f"hbuf{g}", [QH, W], bf16).ap())
        yT.append(nc.dram_tensor(f"yT{g}", [D_MODEL, W], f32, kind="ExternalOutput").ap())

    with tile.TileContext(nc) as tc, ExitStack() as ctx:
        # pools that live across both phases (prefetch targets)
        w2pool = ctx.enter_context(tc.tile_pool(name="w2p", bufs=2))
        hinpool = ctx.enter_context(tc.tile_pool(name="hin", bufs=3))
        pre_strip = {}   # (g, dg, ci) -> prefetched hstrip tile
        pre_w2 = {}      # (g, dg) -> list of prefetched w2 tiles

        # NOTE: batching these group loads into single multi-dim-AP DMAs was
        # tried and is ~15% WORSE end to end: one instruction's descriptors
        # land on one DMA ring, serializing the transfer, and the weight
        # double-buffer can no longer keep the PE fed.  Keep per-row DMAs
        # (parallel rings) but place them on queues whose issue slots are
        # free: the engine that runs the activations must NOT also issue a
        # 16-DMA burst, or the PSUM-WAR chain stalls the PE.
        def load_strip(g, ci, queues):
            n0, sz = chunk_lists[g][ci]
            t = hinpool.tile([P, QT * 512], bf16, tag="hs", name="hs")
            for h in range(QT):
                queues[h % len(queues)].dma_start(
                    out=t[:, h * sz : (h + 1) * sz],
                    in_=hbuf[g][h * P : (h + 1) * P, n0 : n0 + sz],
                )
            return t

        def load_w2(g, dg):
            t = w2pool.tile([P, QT * D_SPAN], bf16, tag="w2g", name="w2g")
            for h in range(QT):
                nc.sync.dma_start(
                    out=t[:, h * D_SPAN : (h + 1) * D_SPAN],
                    in_=w2[g][h * P : (h + 1) * P, dg * D_SPAN : (dg + 1) * D_SPAN],
                )
            return t

        # ---- phase 1: per slot, H^T = silu(W1q^T x^T) * (W3q^T x^T) -> hbuf
        with ExitStack() as p1:
            xpool = p1.enter_context(tc.tile_pool(name="xp", bufs=2))
            wpool = p1.enter_context(tc.tile_pool(name="w13", bufs=2))
            pspool = p1.enter_context(tc.tile_pool(name="ps1", bufs=1, space="PSUM"))
            spool = p1.enter_context(tc.tile_pool(name="sg", bufs=4))
            hpool = p1.enter_context(tc.tile_pool(name="hout", bufs=3))

            def alloc_x():
                return [
                    xpool.tile([P, W0], bf16, tag=f"x{c}", name=f"x{c}")
                    for c in range(DC)
                ]

            def load_x_piece(ts, g, c):
                W = widths[g]
                nc.gpsimd.dma_start(
                    out=ts[c][:, :W], in_=xT[g][c * P : (c + 1) * P, :]
                )

            # PE warm-up: ~72 junk matmuls on a memset tile during the initial
            # DMA wait so HAM un-throttles (K=8/8) before the first real MM.
            # The result lands in hbuf0[0:P, 0:64], which the real h-row 0
            # write fully overwrites (WAW keeps it ordered and live).
            wsrc = spool.tile([P, P], bf16, tag="warm_src", name="wsrc")
            nc.vector.memset(wsrc[:], 0.0)
            wps = pspool.tile([P, 512], f32, tag="pg0", bufs=2, name="warm_ps")
            NWARM = 72
            for i in range(NWARM):
                nc.tensor.matmul(
                    wps[:, :64], wsrc[:], wsrc[:, :64],
                    start=(i == 0), stop=(i == NWARM - 1),
                )
            wsb = spool.tile([P, 64], bf16, tag="warm_sb", name="wsb")
            nc.scalar.copy(wsb[:], wps[:, :64])
            nc.gpsimd.dma_start(out=hbuf[0][0:P, 0:64], in_=wsb[:])

            xts = None
            xts_next = None
            for g in range(4):
                W = widths[g]
                chunks = chunk_lists[g]
                nchunks = len(chunks)
                for gi, (t0, nt) in enumerate(HGROUPS):
                    span = nt * P
                    h0 = t0 * P
                    w1g = wpool.tile([P, DC * 256], bf16, tag="w1g", name="w1g")
                    w3g = wpool.tile([P, DC * 256], bf16, tag="w3g", name="w3g")
                    if g == 0 and gi == 0:
                        # startup: per-c DMAs for fine-grained deps.  The
                        # first 4 x tiles ride the scalar queue AHEAD of the
                        # w3 batch (x is needed from ~11us, w3 only from
                        # ~18us); these loads have no unsatisfied waits, so
                        # they cannot block the queue.  Rest of x on gpsimd.
                        xts = []
                        for c in range(DC):
                            t = xpool.tile([P, W0], bf16, tag=f"x{c}", name=f"x{c}")
                            xq = nc.scalar if c < 4 else nc.gpsimd
                            xq.dma_start(
                                out=t[:, :W], in_=xT[0][c * P : (c + 1) * P, :]
                            )
                            xts.append(t)
                        for c in range(DC):
                            nc.sync.dma_start(
                                out=w1g[:, c * span : (c + 1) * span],
                                in_=w1[g][c * P : (c + 1) * P, h0 : h0 + span],
                            )
                            nc.scalar.dma_start(
                                out=w3g[:, c * span : (c + 1) * span],
                                in_=w3[g][c * P : (c + 1) * P, h0 : h0 + span],
                            )
                    else:
                        # w1 on sync, w3 on gpsimd: scalar carries ONLY the
                        # silu activations in steady state, so the PE's
                        # PSUM-WAR wait on them never queues behind DMA issue.
                        for c in range(DC):
                            nc.sync.dma_start(
                                out=w1g[:, c * span : (c + 1) * span],
                                in_=w1[g][c * P : (c + 1) * P, h0 : h0 + span],
                            )
                        for c in range(DC):
                            nc.gpsimd.dma_start(
                                out=w3g[:, c * span : (c + 1) * span],
                                in_=w3[g][c * P : (c + 1) * P, h0 : h0 + span],
                            )
                    if gi == 0 and g < 3:
                        # allocate next slot's x tiles; loads are paced 2 per
                        # h-row below (a single 4.3MB burst starves the weight
                        # stream's DMA bandwidth and stalls the PE)
                        xts_next = alloc_x()
                    for hl in range(nt):
                        hrow = t0 + hl
                        htile = hpool.tile([P, W0], bf16, tag="ht", name="ht")
                        pgs = [
                            pspool.tile(
                                [P, 512], f32, tag=f"pg{j}",
                                bufs=(2 if j == 0 else 1), name=f"pg{j}",
                            )
                            for j in range(nchunks)
                        ]
                        pvs = [
                            pspool.tile([P, 512], f32, tag=f"pv{j}", name=f"pv{j}")
                            for j in range(nchunks)
                        ]
                        for c in range(DC):
                            lhs = w1g[:, c * span + hl * P : c * span + hl * P + P]
                            for j, (n0, sz) in enumerate(chunks):
                                nc.tensor.matmul(
                                    pgs[j][:, :sz],
                                    lhs,
                                    xts[c][:, n0 : n0 + sz],
                                    start=(c == 0),
                                    stop=(c == DC - 1),
                                )
                        for c in range(DC):
                            lhs = w3g[:, c * span + hl * P : c * span + hl * P + P]
                            for j, (n0, sz) in enumerate(chunks):
                                nc.tensor.matmul(
                                    pvs[j][:, :sz],
                                    lhs,
                                    xts[c][:, n0 : n0 + sz],
                                    start=(c == 0),
                                    stop=(c == DC - 1),
                                )
                        for j, (n0, sz) in enumerate(chunks):
                            sg_t = spool.tile([P, 512], f32, tag="sg", name="sg_t")
                            nc.scalar.activation(sg_t[:, :sz], pgs[j][:, :sz], Silu)
                            nc.vector.tensor_tensor(
                                htile[:, n0 : n0 + sz], sg_t[:, :sz],
                                pvs[j][:, :sz], op=mult,
                            )
                        nc.sync.dma_start(
                            out=hbuf[g][hrow * P : (hrow + 1) * P, :W],
                            in_=htile[:, :W],
                        )
                        if xts_next is not None and hrow < DC // 2:
                            load_x_piece(xts_next, g + 1, 2 * hrow)
                            load_x_piece(xts_next, g + 1, 2 * hrow + 1)
                # end of slot g: cross-phase prefetches
                # Cross-phase prefetches go on sync at g==2: by then all of
                # slot-0's h-stores are long done, so these DMAs never sit in
                # a queue holding an unsatisfied wait.  (On scalar they block
                # the silu activations and stall the PE via the PSUM WAR.)
                if g == 2:
                    pre_strip[(0, 0, 0)] = load_strip(0, 0, [nc.sync])
                    pre_w2[(0, 0)] = load_w2(0, 0)
                xts = xts_next
                xts_next = None

        # ---- phase 2: per slot, y^T_partial = W2q^T H^T
        with ExitStack() as p2:
            ps2 = p2.enter_context(tc.tile_pool(name="ps2", bufs=2, space="PSUM"))
            ypool = p2.enter_context(tc.tile_pool(name="yst", bufs=4))

            DG = D_MODEL // D_SPAN
            DTS = D_SPAN // P
            # software pipeline: strips issued 2 chunks ahead (bufs=3 ring),
            # each dg's w2 issued during the previous dg's second chunk, so
            # dg/slot boundaries never wait on DMA.
            triples = [
                (g, dg, ci)
                for g in range(4)
                for dg in range(DG)
                for ci in range(len(chunk_lists[g]))
            ]
            pairs = [(g, dg) for g in range(4) for dg in range(DG)]
            strips = dict(pre_strip)
            w2s = dict(pre_w2)
            next_w2 = 0 if (0, 0) not in w2s else 1

            def ensure_strip(i, queues):
                if i < len(triples) and triples[i] not in strips:
                    g_, dg_, ci_ = triples[i]
                    strips[triples[i]] = load_strip(g_, ci_, queues)

            # Strips go on gpsimd ONLY.  Scalar must carry zero DMAs: the
            # tile scheduler may hoist a DMA far ahead of its program-order
            # position on the scalar queue, and an unsatisfied semaphore wait
            # there blocks the silu activations -> 10us+ PE stall via the
            # PSUM WAR chain (observed twice).
            strip_q = [nc.gpsimd]
            ensure_strip(0, strip_q)
            ensure_strip(1, strip_q)
            for idx, (g, dg, ci) in enumerate(triples):
                chunks = chunk_lists[g]
                d0 = dg * D_SPAN
                n0, sz = chunks[ci]
                if (g, dg) not in w2s:
                    w2s[(g, dg)] = load_w2(g, dg)
                    next_w2 = pairs.index((g, dg)) + 1
                w2g = w2s[(g, dg)]
                hstrip = strips[(g, dg, ci)]
                ensure_strip(idx + 2, strip_q)
                if ci == 1 and next_w2 < len(pairs):
                    w2s[pairs[next_w2]] = load_w2(*pairs[next_w2])
                    next_w2 += 1
                final_triple = idx == len(triples) - 1
                for half in range(DTS // DT_SWEEP):
                    ps = [
                        ps2.tile([P, 512], f32, tag=f"yp{q}", name=f"yp{q}")
                        for q in range(DT_SWEEP)
                    ]
                    final_half = final_triple and half == DTS // DT_SWEEP - 1
                    if final_half:
                        # q-major on the very last sweep: each accumulation
                        # group finishes early so its copy+store overlaps the
                        # remaining matmuls instead of trailing the kernel.
                        for q in range(DT_SWEEP):
                            dt = half * DT_SWEEP + q
                            for h in range(QT):
                                nc.tensor.matmul(
                                    ps[q][:, :sz],
                                    w2g[:, h * D_SPAN + dt * P : h * D_SPAN + (dt + 1) * P],
                                    hstrip[:, h * sz : (h + 1) * sz],
                                    start=(h == 0),
                                    stop=(h == QT - 1),
                                )
                            yst = ypool.tile([P, 512], f32, tag="yst", name="yst")
                            nc.scalar.copy(yst[:, :sz], ps[q][:, :sz])
                            nc.sync.dma_start(
                                out=yT[g][
                                    d0 + dt * P : d0 + (dt + 1) * P, n0 : n0 + sz
                                ],
                                in_=yst[:, :sz],
                            )
                        continue
                    for h in range(QT):
                        for q in range(DT_SWEEP):
                            dt = half * DT_SWEEP + q
                            nc.tensor.matmul(
                                ps[q][:, :sz],
                                w2g[:, h * D_SPAN + dt * P : h * D_SPAN + (dt + 1) * P],
                                hstrip[:, h * sz : (h + 1) * sz],
                                start=(h == 0),
                                stop=(h == QT - 1),
                            )
                    for q in range(DT_SWEEP):
                        dt = half * DT_SWEEP + q
                        yst = ypool.tile([P, 512], f32, tag="yst", name="yst")
                        nc.scalar.copy(yst[:, :sz], ps[q][:, :sz])
                        nc.sync.dma_start(
                            out=yT[g][
                                d0 + dt * P : d0 + (dt + 1) * P, n0 : n0 + sz
                            ],
                            in_=yst[:, :sz],
                        )

    nc.compile()
    return nc


def _get_program(widths: tuple):
    if widths not in _prog_cache:
        _prog_cache[widths] = _build_program(widths)
    return _prog_cache[widths]


def _route(x2d: np.ndarray, Wg: np.ndarray):
    """Host gate: float64 softmax + top-2 + renormalize."""
    logits = x2d.astype(np.float64) @ Wg.astype(np.float64)
    logits -= logits.max(axis=-1, keepdims=True)
    e = np.exp(logits)
    p = e / e.sum(axis=-1, keepdims=True)
    top = np.argsort(-p, axis=-1, kind="stable")[:, :TOP_K]
    w = np.take_along_axis(p, top, axis=-1)
    w = w / w.sum(axis=-1, keepdims=True)
    return top, w.astype(np.float32)


def _assignment(tok_lists):
    """Experts sorted desc by padded count; pair 2g with 2g+1; slot width =
    heavier member's padded count."""
    npad = [max(((len(t) + 7) // 8) * 8, 24) for t in tok_lists]
    order = sorted(range(N_EXPERTS), key=lambda e: (-npad[e], e))
    widths = tuple(npad[order[2 * g]] for g in range(4))
    return order, widths


def _prepare(inputs: dict):
    x = np.asarray(inputs["x"], dtype=np.float32)
    Wg = np.asarray(inputs["Wg"], dtype=np.float32)
    W1 = np.asarray(inputs["W1"], dtype=np.float32)
    W3 = np.asarray(inputs["W3"], dtype=np.float32)
    W2 = np.asarray(inputs["W2"], dtype=np.float32)

    b, s, d = x.shape
    T = b * s
    x2d = np.ascontiguousarray(x.reshape(T, d))

    top, wts = _route(x2d, Wg)

    tok_lists = []
    wt_lists = []
    for e in range(N_EXPERTS):
        mask = top == e  # [T, K]
        toks = np.where(mask.any(axis=-1))[0]
        we = wts[toks][mask[toks]]  # one weight per selected token
        tok_lists.append(toks)
        wt_lists.append(we.astype(np.float32))

    order, widths = _assignment(tok_lists)
    nc = _get_program(widths)

    W1bf = W1.astype(ml_dtypes.bfloat16)
    W3bf = W3.astype(ml_dtypes.bfloat16)
    W2bf = W2.astype(ml_dtypes.bfloat16)
    x2dbf = x2d.astype(ml_dtypes.bfloat16)

    # one padded x^T buffer per slot-group expert (shared by its 4 cores)
    xTe = {}
    for g in range(4):
        W = widths[g]
        for E in (order[2 * g], order[2 * g + 1]):
            toks = tok_lists[E]
            buf = np.zeros((d, W), dtype=ml_dtypes.bfloat16)
            buf[:, : len(toks)] = x2dbf[toks].T
            xTe[E] = buf

    in_maps = []
    for c in range(N_CORES):
        q = c % 4
        m = {}
        for g in range(4):
            E = order[2 * g] if c < 4 else order[2 * g + 1]
            m[f"xT{g}"] = xTe[E]
            m[f"w1{g}"] = np.ascontiguousarray(W1bf[E][:, q * QH : (q + 1) * QH])
            m[f"w3{g}"] = np.ascontiguousarray(W3bf[E][:, q * QH : (q + 1) * QH])
            m[f"w2{g}"] = np.ascontiguousarray(W2bf[E][q * QH : (q + 1) * QH, :])
        in_maps.append(m)

    return nc, in_maps, tok_lists, wt_lists, (b, s, d)


def _combine(results, tok_lists, wt_lists, shape):
    b, s, d = shape
    order, widths = _assignment(tok_lists)
    out2d = np.zeros((b * s, d), dtype=np.float32)
    for g in range(4):
        for base, E in ((0, order[2 * g]), (4, order[2 * g + 1])):
            toks = tok_lists[E]
            n = len(toks)
            acc = results[base][f"yT{g}"][:, :n].astype(np.float32).copy()
            for q in range(1, 4):
                acc += results[base + q][f"yT{g}"][:, :n]
            out2d[toks] += wt_lists[E][:, None] * acc.T
    return out2d.reshape(b, s, d)


def _ensure_trace_hooks():
    """If BASS_TRACE is set, run_bass_kernel_spmd imports antenv.axon_hooks,
    which some images lack. Provide the standard shim (ctypes into the axon
    .so) when missing, and make the artifact upload failure-tolerant."""
    import sys

    try:
        import antenv.axon_hooks  # noqa: F401
        return
    except ImportError:
        pass
    import contextlib
    import ctypes
    import types

    so_path = "/opt/axon/libaxon_pjrt.so"
    hook = None
    try:
        lib = ctypes.CDLL(so_path)
        lib.axon_start_nrt_profile.argtypes = [
            ctypes.POINTER(ctypes.c_int64),
            ctypes.c_size_t,
        ]
        lib.axon_start_nrt_profile.restype = ctypes.c_int64
        lib.axon_stop_nrt_profile.argtypes = [ctypes.c_char_p]
        lib.axon_stop_nrt_profile.restype = ctypes.c_int64

        @contextlib.contextmanager
        def _hook(output_dir, device_ids):
            import jax

            jax.devices()
            if device_ids:
                ids = (ctypes.c_int64 * len(device_ids))(*device_ids)
                rc = lib.axon_start_nrt_profile(ids, len(device_ids))
            else:
                rc = lib.axon_start_nrt_profile(None, 0)
            if rc != 0:
                raise RuntimeError(f"axon_start_nrt_profile rc={rc}")
            try:
                yield
            finally:
                lib.axon_stop_nrt_profile(str(output_dir).encode())

        hook = _hook
    except Exception:
        hook = None

    mod = types.ModuleType("antenv.axon_hooks")
    state = {"hook": hook}
    mod.get_axon_ntff_profile_hook = lambda: state["hook"]
    mod.set_axon_ntff_profile_hook = lambda h: state.update(hook=h)
    sys.modules["antenv.axon_hooks"] = mod
    try:
        import antenv

        antenv.axon_hooks = mod
    except ImportError:
        pass

    import concourse.bass_utils as bu

    orig_upload = bu.upload_artifacts

    def _safe_upload(tmpdir):
        try:
            return orig_upload(tmpdir)
        except Exception:
            return f"local://{tmpdir}"

    bu.upload_artifacts = _safe_upload


def kernel(**inputs) -> np.ndarray:
    from concourse.bass_utils import run_bass_kernel_spmd

    _ensure_trace_hooks()
    nc, in_maps, tok_lists, wt_lists, shape = _prepare(inputs)
    res = run_bass_kernel_spmd(nc, in_maps, core_ids=list(range(N_CORES)))
    return _combine(res.results, tok_lists, wt_lists, shape)
